# revision 1
# baseline (speedup 1.0000x reference)
"""Trainium2 Bass kernel for a device-aware top-1 MoE layer.

Strategy (expert parallelism over 8 NeuronCores):
  - Host: compute gate logits + top-1 routing (this is the "dispatch"
    step of the sharding), gather each expert's tokens, pad to a common
    capacity C, and transpose to feature-major [D, C] so the device
    matmuls need no on-chip transposes.
  - Device (SPMD, one NEFF on 8 cores): core i holds experts (2i, 2i+1)
    in bf16. For each expert:  hT = relu(w1.T-chunks @ xT + b1),
    yT = w2.T-chunks @ hT + b2, with fp32 PSUM accumulation.
    Activations stay [feature, token] so biases are per-partition.
  - Host: scatter each expert's [D, count] output back to token rows.

Perf notes:
  - Weights are bf16 (fp32 matmul is quarter-rate on the PE and doubles
    HBM traffic; fp32 PSUM accumulation keeps rel err ~3e-3).
  - The kernel is HBM-bound (~18 MB/core at ~360 GB/s/core). All weight
    DMA rides the sync HWDGE queue in exact consumption order; stage 2
    iterates h-outer so the PE consumes w2 tiles as they arrive instead
    of waiting for the whole expert.
  - Bias+relu / bias+copy epilogues alternate between ScalarE and
    VectorE so neither engine becomes the drain bottleneck.
"""

import numpy as np
import ml_dtypes

D = 1024
H = 2048
E = 16
NCORES = 8
P = 128
DB = D // P   # 8 d-chunks
HB = H // P   # 16 h-chunks

_program_cache = {}


def _build_program(C):
    """Trace the per-core Bass/Tile program for token capacity C (<=512)."""
    import concourse.tile as tile
    from concourse import bacc, mybir

    assert C <= 512
    f32 = mybir.dt.float32
    bf16 = mybir.dt.bfloat16
    AF = mybir.ActivationFunctionType
    ALU = mybir.AluOpType

    nc = bacc.Bacc(
        "TRN2", target_bir_lowering=False, debug=False, num_devices=NCORES
    )
    xT = nc.dram_tensor("xT", [D, 2 * C], bf16, kind="ExternalInput").ap()
    w1s = nc.dram_tensor("w1s", [2, D, H], bf16, kind="ExternalInput").ap()
    w2s = nc.dram_tensor("w2s", [2, H, D], bf16, kind="ExternalInput").ap()
    b1s = nc.dram_tensor("b1s", [2, P, HB], f32, kind="ExternalInput").ap()
    b2s = nc.dram_tensor("b2s", [2, P, DB], f32, kind="ExternalInput").ap()
    yT = nc.dram_tensor("yT", [2, D, C], bf16, kind="ExternalOutput").ap()

    with tile.TileContext(nc) as tc:
        with (
            tc.tile_pool(name="xp", bufs=2) as xp,
            tc.tile_pool(name="w1p", bufs=8) as w1p,
            tc.tile_pool(name="w2p", bufs=8) as w2p,
            tc.tile_pool(name="hp", bufs=32) as hp,
            tc.tile_pool(name="bp", bufs=4) as bp,
            tc.tile_pool(name="yp", bufs=8) as yp,
            tc.tile_pool(name="ps", bufs=8, space="PSUM") as ps,
        ):
            xts = [None, None]
            hts = [[None] * HB for _ in range(2)]
            yts = []
            b1ts = [None, None]
            b2ts = [None, None]

            # Input DMAs in consumption order on the sync HWDGE queue,
            # batched into ~1-2MB transfers (fewer triggers, deeper
            # in-flight pipelining). Tiny bias tiles go via gpsimd.
            xT3 = xT.rearrange("(o p) c -> p o c", p=P)        # [128, 8, 2C]
            w13 = [
                w1s[e].rearrange("(o p) h -> p o h", p=P) for e in range(2)
            ]                                                   # [128, 8, H]
            w23 = [
                w2s[e].rearrange("(o p) f -> p o f", p=P) for e in range(2)
            ]                                                   # [128, 16, D]

            for e in range(2):
                b1t = bp.tile([P, HB], f32, tag="b1")
                nc.gpsimd.dma_start(b1t[:], b1s[e])
                b1ts[e] = b1t
                b2t = bp.tile([P, DB], f32, tag="b2")
                nc.gpsimd.dma_start(b2t[:], b2s[e])
                b2ts[e] = b2t

            def epilogue(i, out_t, acc_t, bias_col, relu):
                """Bias (+relu) from PSUM to SBUF, alternating engines."""
                if i % 2 == 0:
                    nc.scalar.activation(
                        out_t[:], acc_t[:],
                        AF.Relu if relu else AF.Identity,
                        bias=bias_col,
                    )
                elif relu:
                    nc.vector.tensor_scalar(
                        out_t[:], acc_t[:], bias_col, 0.0, ALU.add, ALU.max
                    )
                else:
                    nc.vector.tensor_scalar_add(out_t[:], acc_t[:], bias_col)

            W1G = 2   # d-chunks per w1 DMA (1MB)
            W2G = 4   # h-chunks per w2 DMA (1MB)
            for e in range(2):
                # xT + w1 for this expert (queue position: after the
                # previous expert's w2, matching PE consumption order).
                xt = xp.tile([P, DB, C], bf16, tag="xT")
                nc.sync.dma_start(xt[:], xT3[:, :, e * C:(e + 1) * C])
                xts[e] = xt
                w1ts = []
                for g in range(DB // W1G):
                    w1t = w1p.tile([P, W1G, H], bf16, tag="w1")
                    nc.sync.dma_start(
                        w1t[:], w13[e][:, g * W1G:(g + 1) * W1G, :]
                    )
                    w1ts.append(w1t)

                # ---- stage 1: hT = relu(w1.T @ xT + b1) ----
                if e == 0:
                    # Expert 0's w1 load gates the PE start: two half-H
                    # passes, d-outer within each, so the PE starts on the
                    # first w1 d-chunks as soon as their DMAs land (the
                    # second pass reuses the then-resident w1 tiles).
                    for half in range(2):
                        accs1 = [
                            ps.tile([P, C], f32, tag="acc",
                                    name=f"acc1_{e}_{half}_{i}")
                            for i in range(HB // 2)
                        ]
                        for d in range(DB):
                            for hh in range(HB // 2):
                                h = half * (HB // 2) + hh
                                nc.tensor.matmul(
                                    accs1[hh][:],
                                    lhsT=w1ts[d // W1G][:, d % W1G, h * P:(h + 1) * P],
                                    rhs=xts[e][:, d, :],
                                    start=(d == 0),
                                    stop=(d == DB - 1),
                                )
                        for hh in range(HB // 2):
                            h = half * (HB // 2) + hh
                            ht = hp.tile([P, C], bf16, tag="hT")
                            epilogue(
                                h, ht, accs1[hh], b1ts[e][:, h:h + 1],
                                relu=True,
                            )
                            hts[e][h] = ht
                else:
                    # Expert 1's w1 is resident by the time the PE gets
                    # here: h-outer retires each psum right away, so the
                    # epilogues pipeline with the next chain's matmuls.
                    for h in range(HB):
                        acc = ps.tile([P, C], f32, tag="acc")
                        for d in range(DB):
                            nc.tensor.matmul(
                                acc[:],
                                lhsT=w1ts[d // W1G][:, d % W1G, h * P:(h + 1) * P],
                                rhs=xts[e][:, d, :],
                                start=(d == 0),
                                stop=(d == DB - 1),
                            )
                        ht = hp.tile([P, C], bf16, tag="hT")
                        epilogue(h, ht, acc, b1ts[e][:, h:h + 1], relu=True)
                        hts[e][h] = ht

                # ---- stage 2: yT = w2.T @ hT + b2 (h-outer so the PE
                # consumes each w2 tile as soon as its DMA lands) ----
                accs = [
                    ps.tile([P, C], f32, tag="acc", name=f"acc2_{e}_{d}")
                    for d in range(DB)
                ]
                for g in range(HB // W2G):
                    w2t = w2p.tile([P, W2G, D], bf16, tag="w2")
                    nc.sync.dma_start(
                        w2t[:], w23[e][:, g * W2G:(g + 1) * W2G, :]
                    )
                    for hh in range(W2G):
                        h = g * W2G + hh
                        for d in range(DB):
                            nc.tensor.matmul(
                                accs[d][:],
                                lhsT=w2t[:, hh, d * P:(d + 1) * P],
                                rhs=hts[e][h][:],
                                start=(h == 0),
                                stop=(h == HB - 1),
                            )
                yt = yp.tile([P, DB, C], bf16, tag="yt")
                for d in range(DB):
                    epilogue(
                        d, yt[:, d, :], accs[d], b2ts[e][:, d:d + 1],
                        relu=False,
                    )
                yts.append((e, yt))

            # Output writes LAST on the sync queue: the 8 HWDGE queue
            # semaphores are shared across engines, so a compute-gated
            # write queued before a weight load would head-of-line block
            # the load's trigger pacing. One batched DMA per expert.
            yT3 = yT.rearrange("e (o p) c -> e p o c", p=P)
            for e, yt in yts:
                (nc.gpsimd if e == 0 else nc.sync).dma_start(yT3[e], yt[:])

    nc.compile()
    return nc


def kernel(x, gate_w, gate_b, w1, b1, w2, b2, _trace=False):
    from concourse.bass_utils import run_bass_kernel_spmd

    x = np.asarray(x, dtype=np.float32)
    B, S, d_in = x.shape
    T = B * S
    xf = x.reshape(T, d_in)

    # --- routing (host side: this is the dispatch/sharding step) ---
    logits = xf @ np.asarray(gate_w, dtype=np.float32) + np.asarray(
        gate_b, dtype=np.float32
    )
    top1 = np.argmax(logits, axis=-1)
    idxs = [np.nonzero(top1 == e)[0] for e in range(E)]
    C = max(32, max(len(i) for i in idxs))
    C = (C + 3) // 4 * 4
    C = min(C, 512)
    assert all(len(i) <= C for i in idxs), "expert capacity overflow"

    if C not in _program_cache:
        _program_cache[C] = _build_program(C)
    nc = _program_cache[C]

    bf16 = ml_dtypes.bfloat16
    w1 = np.asarray(w1)
    w2 = np.asarray(w2)
    b1 = np.asarray(b1, dtype=np.float32)
    b2 = np.asarray(b2, dtype=np.float32)

    in_maps = []
    for core in range(NCORES):
        xT = np.zeros((D, 2 * C), dtype=bf16)
        w1s = np.empty((2, D, H), dtype=bf16)
        w2s = np.empty((2, H, D), dtype=bf16)
        b1s = np.empty((2, P, HB), dtype=np.float32)
        b2s = np.empty((2, P, DB), dtype=np.float32)
        for s in range(2):
            e = 2 * core + s
            idx = idxs[e]
            if len(idx):
                xT[:, s * C:s * C + len(idx)] = xf[idx].T.astype(bf16)
            w1s[s] = w1[e].astype(bf16)
            w2s[s] = w2[e].astype(bf16)
            b1s[s] = b1[e].reshape(HB, P).T
            b2s[s] = b2[e].reshape(DB, P).T
        in_maps.append(
            {"xT": xT, "w1s": w1s, "w2s": w2s, "b1s": b1s, "b2s": b2s}
        )

    res = run_bass_kernel_spmd(
        nc, in_maps, core_ids=list(range(NCORES)), trace=_trace
    )

    out = np.zeros((T, D), dtype=np.float32)
    for core in range(NCORES):
        yT_out = res.results[core]["yT"]
        for s in range(2):
            e = 2 * core + s
            idx = idxs[e]
            if len(idx):
                out[idx] = yT_out[s][:, :len(idx)].T.astype(np.float32)
    if _trace:
        kernel.last_result = res
    return out.reshape(B, S, D)



# revision 2
# speedup vs baseline: 1.3031x; 1.3031x over previous
"""Trainium2 Bass kernel for a device-aware top-1 MoE layer.

Strategy (expert parallelism over 8 NeuronCores):
  - Host: gate + top-1 routing, then pack each expert's tokens.
    Experts are paired big+small across cores (sorted by count) so the
    program's two capacity slots (C1 >= C2) waste little padding.
  - Device (SPMD, one NEFF on 8 cores): core holds 2 experts in fp8
    e3m4 (weights pre-scaled by 2^8 / 2^9 so they sit in e3m4's normal
    range; power-of-two scales are exact).  Activations are fp16.
      stage 1: h' = relu(w1q.T @ xT + 2^8*b1)     (= 2^8 * h, exact)
      stage 2: y  = (w2q.T @ h') * 2^-17 + b2     (epilogue scale)
    fp32 PSUM accumulation throughout.
  - Host: scatter each expert's [D, count] output back to token rows.

Perf notes:
  - fp8 weights halve HBM traffic vs bf16 (8 MB/core weights) AND keep
    the PE fed: the PE consumes fp8 weights at ~250 GB/s < 358 GB/s DMA,
    so the tensor engine (not DMA) sets the pace after warmup.
  - Every DRAM tensor is host-packed into its exact SBUF image
    [128, bytes] so all DMA descriptors are long contiguous lines
    (2-16 KB), maximizing HBM efficiency.
  - All weight DMA rides the sync HWDGE queue in exact consumption
    order; stage 2 iterates h-outer so the PE consumes w2 tiles as they
    arrive.  Slot-0 output is written early on the gpsimd queue to
    overlap slot-1 compute.
"""

import numpy as np
import ml_dtypes

D = 1024
H = 2048
E = 16
NCORES = 8
P = 128
DB = D // P   # 8 d-chunks
HB = H // P   # 16 h-chunks
W1G = 2       # d-chunks per w1 DMA (0.5 MB)
W2G = 4       # h-chunks per w2 DMA (0.5 MB)
S1 = 256.0    # 2^8  w1 scale
S2 = 512.0    # 2^9  w2 scale
UNSCALE = 1.0 / (S1 * S2)

_program_cache = {}


def _build_program(C1, C2):
    """Trace the per-core Bass/Tile program for capacities (C1, C2)."""
    import concourse.tile as tile
    from concourse import bacc, mybir

    assert C1 <= 512 and C2 <= C1
    f32 = mybir.dt.float32
    f16 = mybir.dt.float16
    f8 = mybir.dt.float8e3
    AF = mybir.ActivationFunctionType
    ALU = mybir.AluOpType
    CS = (C1, C2)

    nc = bacc.Bacc(
        "TRN2", target_bir_lowering=False, debug=False, num_devices=NCORES
    )
    xT = nc.dram_tensor("xT", [P, DB * (C1 + C2)], f16, kind="ExternalInput").ap()
    w1s = nc.dram_tensor("w1s", [2, P, DB * H], f8, kind="ExternalInput").ap()
    w2s = nc.dram_tensor("w2s", [2, P, HB * D], f8, kind="ExternalInput").ap()
    b1s = nc.dram_tensor("b1s", [2, P, HB], f32, kind="ExternalInput").ap()
    b2s = nc.dram_tensor("b2s", [2, P, DB], f32, kind="ExternalInput").ap()
    y0 = nc.dram_tensor("y0", [P, DB * C1], f16, kind="ExternalOutput").ap()
    y1 = nc.dram_tensor("y1", [P, DB * C2], f16, kind="ExternalOutput").ap()
    ys = (y0, y1)

    with tile.TileContext(nc) as tc:
        with (
            tc.tile_pool(name="xp", bufs=2) as xp,
            tc.tile_pool(name="w1p", bufs=8) as w1p,
            tc.tile_pool(name="w2p", bufs=8) as w2p,
            tc.tile_pool(name="hp", bufs=32) as hp,
            tc.tile_pool(name="bp", bufs=4) as bp,
            tc.tile_pool(name="yp", bufs=2) as yp,
            tc.tile_pool(name="ps", bufs=8, space="PSUM") as ps,
        ):
            xts = [None, None]
            hts = [[None] * HB for _ in range(2)]
            b1ts = [None, None]
            b2ts = [None, None]

            # Tiny bias tiles ride the gpsimd queue.
            for s in range(2):
                b1t = bp.tile([P, HB], f32, tag="b1")
                nc.gpsimd.dma_start(b1t[:], b1s[s])
                b1ts[s] = b1t
                b2t = bp.tile([P, DB], f32, tag="b2")
                nc.gpsimd.dma_start(b2t[:], b2s[s])
                b2ts[s] = b2t

            def epi1(i, out_t, acc_t, bias_col):
                """relu(acc + b1s) from PSUM to SBUF, alternating engines."""
                if i % 2 == 0:
                    nc.scalar.activation(out_t[:], acc_t[:], AF.Relu, bias=bias_col)
                else:
                    nc.vector.tensor_scalar(
                        out_t[:], acc_t[:], bias_col, 0.0, ALU.add, ALU.max
                    )

            def epi2(i, out_t, acc_t, bias_col):
                """acc * 2^-17 + b2 from PSUM to SBUF, alternating engines."""
                if i % 2 == 0:
                    nc.scalar.activation(
                        out_t[:], acc_t[:], AF.Identity,
                        bias=bias_col, scale=UNSCALE,
                    )
                else:
                    nc.vector.tensor_scalar(
                        out_t[:], acc_t[:], UNSCALE, bias_col, ALU.mult, ALU.add
                    )

            xoff = [0, DB * C1]
            for s in range(2):
                C = CS[s]
                # x + w1 for this slot, on the sync queue in consumption
                # order (slot 1's loads queue behind slot 0's w2 below).
                xt = xp.tile([P, DB * C], f16, tag="xT")
                nc.sync.dma_start(xt[:], xT[:, xoff[s]:xoff[s] + DB * C])
                xts[s] = xt
                w1ts = []
                for g in range(DB // W1G):
                    w1t = w1p.tile([P, W1G * H], f8, tag="w1")
                    nc.sync.dma_start(
                        w1t[:], w1s[s, :, g * W1G * H:(g + 1) * W1G * H]
                    )
                    w1ts.append(w1t)

                # ---- stage 1: h' = relu(w1q.T @ xT + b1s) ----
                if s == 0:
                    # Slot 0's w1 load gates the PE start: two half-H
                    # passes, d-outer within each, so the PE starts on
                    # the first w1 d-chunks as soon as their DMAs land.
                    for half in range(2):
                        accs1 = [
                            ps.tile([P, C], f32, tag="acc",
                                    name=f"acc1_{s}_{half}_{i}")
                            for i in range(HB // 2)
                        ]
                        for d in range(DB):
                            for hh in range(HB // 2):
                                h = half * (HB // 2) + hh
                                w1t = w1ts[d // W1G]
                                col = (d % W1G) * H + h * P
                                nc.tensor.matmul(
                                    accs1[hh][:],
                                    lhsT=w1t[:, col:col + P],
                                    rhs=xts[s][:, d * C:(d + 1) * C],
                                    start=(d == 0),
                                    stop=(d == DB - 1),
                                )
                        for hh in range(HB // 2):
                            h = half * (HB // 2) + hh
                            ht = hp.tile([P, C], f16, tag="hT")
                            epi1(h, ht, accs1[hh], b1ts[s][:, h:h + 1])
                            hts[s][h] = ht
                else:
                    # Slot 1's w1 is resident by now: h-outer retires
                    # each psum right away so epilogues pipeline.
                    for h in range(HB):
                        acc = ps.tile([P, C], f32, tag="acc")
                        for d in range(DB):
                            w1t = w1ts[d // W1G]
                            col = (d % W1G) * H + h * P
                            nc.tensor.matmul(
                                acc[:],
                                lhsT=w1t[:, col:col + P],
                                rhs=xts[s][:, d * C:(d + 1) * C],
                                start=(d == 0),
                                stop=(d == DB - 1),
                            )
                        ht = hp.tile([P, C], f16, tag="hT")
                        epi1(h, ht, acc, b1ts[s][:, h:h + 1])
                        hts[s][h] = ht

                # ---- stage 2: y = (w2q.T @ h') * 2^-17 + b2, h-outer so
                # the PE consumes each w2 tile as soon as its DMA lands --
                accs = [
                    ps.tile([P, C], f32, tag="acc", name=f"acc2_{s}_{d}")
                    for d in range(DB)
                ]
                for g in range(HB // W2G):
                    w2t = w2p.tile([P, W2G * D], f8, tag="w2")
                    nc.sync.dma_start(
                        w2t[:], w2s[s, :, g * W2G * D:(g + 1) * W2G * D]
                    )
                    for hh in range(W2G):
                        h = g * W2G + hh
                        for d in range(DB):
                            nc.tensor.matmul(
                                accs[d][:],
                                lhsT=w2t[:, hh * D + d * P:hh * D + d * P + P],
                                rhs=hts[s][h][:],
                                start=(h == 0),
                                stop=(h == HB - 1),
                            )
                yt = yp.tile([P, DB * C], f16, tag="yt")
                for d in range(DB):
                    epi2(d, yt[:, d * C:(d + 1) * C], accs[d],
                         b2ts[s][:, d:d + 1])
                # Slot 0's output write rides the (otherwise idle) gpsimd
                # queue so it overlaps slot 1 compute; slot 1's is last
                # on the sync queue where it can't head-of-line block.
                (nc.gpsimd if s == 0 else nc.sync).dma_start(ys[s], yt[:])

    nc.compile()
    return nc


def kernel(x, gate_w, gate_b, w1, b1, w2, b2, _trace=False):
    from concourse.bass_utils import run_bass_kernel_spmd

    f16 = np.float16
    e3m4 = ml_dtypes.float8_e3m4

    x = np.asarray(x, dtype=np.float32)
    B, S, d_in = x.shape
    T = B * S
    xf = x.reshape(T, d_in)

    # --- routing (host side: this is the dispatch/sharding step) ---
    logits = xf @ np.asarray(gate_w, dtype=np.float32) + np.asarray(
        gate_b, dtype=np.float32
    )
    top1 = np.argmax(logits, axis=-1)
    idxs = [np.nonzero(top1 == e)[0] for e in range(E)]
    counts = np.array([len(i) for i in idxs])

    # Pair big+small experts per core: slot 0 gets the 8 largest.
    order = np.argsort(-counts, kind="stable")
    slot_experts = [
        (int(order[core]), int(order[2 * NCORES - 1 - core]))
        for core in range(NCORES)
    ]

    def cap(n):
        return min(512, max(16, (n + 3) // 4 * 4))

    C1 = cap(max(counts[e0] for e0, _ in slot_experts))
    C2 = cap(max(counts[e1] for _, e1 in slot_experts))
    assert all(counts[a] <= C1 and counts[b] <= C2 for a, b in slot_experts)

    if (C1, C2) not in _program_cache:
        _program_cache[(C1, C2)] = _build_program(C1, C2)
    nc = _program_cache[(C1, C2)]

    # Pre-quantize all expert weights into their SBUF image layouts.
    w1q = (np.asarray(w1, dtype=np.float32) * S1).astype(e3m4)   # [E, D, H]
    w2q = (np.asarray(w2, dtype=np.float32) * S2).astype(e3m4)   # [E, H, D]
    b1f = np.asarray(b1, dtype=np.float32) * S1
    b2f = np.asarray(b2, dtype=np.float32)
    xf16 = xf.astype(f16)
    CS = (C1, C2)

    in_maps = []
    for core in range(NCORES):
        xT = np.zeros((P, DB * (C1 + C2)), dtype=f16)
        w1sv = np.empty((2, P, DB * H), dtype=e3m4)
        w2sv = np.empty((2, P, HB * D), dtype=e3m4)
        b1sv = np.empty((2, P, HB), dtype=np.float32)
        b2sv = np.empty((2, P, DB), dtype=np.float32)
        xoff = (0, DB * C1)
        for s in range(2):
            e = slot_experts[core][s]
            C = CS[s]
            idx = idxs[e]
            n = len(idx)
            if n:
                # [p, dd*C + c] = x[idx[c], dd*128 + p]
                xs = xf16[idx].T.reshape(DB, P, n).transpose(1, 0, 2)
                xv = xT[:, xoff[s]:xoff[s] + DB * C].reshape(P, DB, C)
                xv[:, :, :n] = xs
            # [p, dd*H + h] = w1q[e][dd*128 + p, h]
            w1sv[s] = (
                w1q[e].reshape(DB, P, H).transpose(1, 0, 2).reshape(P, DB * H)
            )
            # [p, oo*D + d] = w2q[e][oo*128 + p, d]
            w2sv[s] = (
                w2q[e].reshape(HB, P, D).transpose(1, 0, 2).reshape(P, HB * D)
            )
            b1sv[s] = b1f[e].reshape(HB, P).T
            b2sv[s] = b2f[e].reshape(DB, P).T
        in_maps.append(
            {"xT": xT, "w1s": w1sv, "w2s": w2sv, "b1s": b1sv, "b2s": b2sv}
        )

    res = run_bass_kernel_spmd(
        nc, in_maps, core_ids=list(range(NCORES)), trace=_trace
    )

    out = np.zeros((T, D), dtype=np.float32)
    for core in range(NCORES):
        for s, yname in ((0, "y0"), (1, "y1")):
            e = slot_experts[core][s]
            C = CS[s]
            idx = idxs[e]
            n = len(idx)
            if n:
                yv = res.results[core][yname]
                yd = (
                    yv.reshape(P, DB, C).transpose(1, 0, 2).reshape(D, C)
                )
                out[idx] = yd[:, :n].T.astype(np.float32)
    if _trace:
        kernel.last_result = res
    return out.reshape(B, S, D)


# revision 4
# speedup vs baseline: 1.3926x; 1.0687x over previous
"""Trainium2 Bass kernel for a device-aware top-1 MoE layer.

Strategy (expert parallelism over 8 NeuronCores):
  - Host: gate + top-1 routing, then pack each expert's tokens.
    Experts are paired big+small across cores (sorted by count) so the
    program's two capacity slots (C1 >= C2) waste little padding.
  - Device (SPMD, one NEFF on 8 cores): core holds 2 experts in fp8
    e3m4 (weights pre-scaled by 2^8 / 2^9 so they sit in e3m4's normal
    range; power-of-two scales are exact).  Activations are fp16.
      stage 1: h' = relu(w1q.T @ xT + 2^8*b1)     (= 2^8 * h, exact)
      stage 2: y  = (w2q.T @ h') * 2^-17 + b2     (epilogue scale)
    fp32 PSUM accumulation throughout.
  - Host: scatter each expert's [D, count] output back to token rows.

Perf notes:
  - fp8 weights halve HBM traffic vs bf16 (8 MB/core weights) AND keep
    the PE fed: the PE consumes fp8 weights at ~250 GB/s < 358 GB/s DMA,
    so the tensor engine (not DMA) sets the pace after warmup.
  - Every DRAM tensor is host-packed into its exact SBUF image
    [128, bytes] so all DMA descriptors are long contiguous lines
    (2-16 KB), maximizing HBM efficiency.
  - All weight DMA rides the sync HWDGE queue in exact consumption
    order; stage 2 iterates h-outer so the PE consumes w2 tiles as they
    arrive.  Slot-0 output is written early on the gpsimd queue to
    overlap slot-1 compute.
"""

import numpy as np
import ml_dtypes

D = 1024
H = 2048
E = 16
NCORES = 8
P = 128
DB = D // P   # 8 d-chunks
HB = H // P   # 16 h-chunks
W1G = 2       # d-chunks per w1 DMA (0.5 MB)
W2G = 4       # h-chunks per w2 DMA (0.5 MB)
S1 = 256.0    # 2^8  w1 scale
S2 = 512.0    # 2^9  w2 scale
UNSCALE = 1.0 / (S1 * S2)

_program_cache = {}


def _build_program(C1, C2):
    """Trace the per-core Bass/Tile program for capacities (C1, C2)."""
    import concourse.tile as tile
    from concourse import bacc, mybir

    assert C1 <= 512 and C2 <= C1
    f32 = mybir.dt.float32
    f16 = mybir.dt.float16
    f8 = mybir.dt.float8e3
    AF = mybir.ActivationFunctionType
    ALU = mybir.AluOpType
    CS = (C1, C2)

    nc = bacc.Bacc(
        "TRN2", target_bir_lowering=False, debug=False, num_devices=NCORES
    )
    xT = nc.dram_tensor("xT", [P, DB * (C1 + C2)], f16, kind="ExternalInput").ap()
    w1s = nc.dram_tensor("w1s", [2, P, DB * H], f8, kind="ExternalInput").ap()
    w2s = nc.dram_tensor("w2s", [2, P, HB * D], f8, kind="ExternalInput").ap()
    b1s = nc.dram_tensor("b1s", [2, P, HB], f32, kind="ExternalInput").ap()
    b2s = nc.dram_tensor("b2s", [2, P, DB], f32, kind="ExternalInput").ap()
    y0 = nc.dram_tensor("y0", [P, DB * C1], f16, kind="ExternalOutput").ap()
    y1 = nc.dram_tensor("y1", [P, DB * C2], f16, kind="ExternalOutput").ap()
    ys = (y0, y1)

    with tile.TileContext(nc) as tc:
        with (
            tc.tile_pool(name="xp", bufs=3) as xp,
            tc.tile_pool(name="w1p", bufs=12) as w1p,
            tc.tile_pool(name="w2p", bufs=8) as w2p,
            tc.tile_pool(name="hp", bufs=32) as hp,
            tc.tile_pool(name="bp", bufs=4) as bp,
            tc.tile_pool(name="yp", bufs=2) as yp,
            tc.tile_pool(name="ps", bufs=8, space="PSUM") as ps,
        ):
            xts = [None, None]
            hts = [[None] * HB for _ in range(2)]
            b1ts = [None, None]
            b2ts = [None, None]

            # Tiny bias tiles ride the gpsimd queue.
            for s in range(2):
                b1t = bp.tile([P, HB], f32, tag="b1")
                nc.gpsimd.dma_start(b1t[:], b1s[s])
                b1ts[s] = b1t
                b2t = bp.tile([P, DB], f32, tag="b2")
                nc.gpsimd.dma_start(b2t[:], b2s[s])
                b2ts[s] = b2t

            def epi1(i, out_t, acc_t, bias_col):
                """relu(acc + b1s) from PSUM to SBUF, alternating engines."""
                if i % 2 == 0:
                    nc.scalar.activation(out_t[:], acc_t[:], AF.Relu, bias=bias_col)
                else:
                    nc.vector.tensor_scalar(
                        out_t[:], acc_t[:], bias_col, 0.0, ALU.add, ALU.max
                    )

            def epi2(i, out_t, acc_t, bias_col):
                """acc * 2^-17 + b2 from PSUM to SBUF, alternating engines."""
                if i % 2 == 0:
                    nc.scalar.activation(
                        out_t[:], acc_t[:], AF.Identity,
                        bias=bias_col, scale=UNSCALE,
                    )
                else:
                    nc.vector.tensor_scalar(
                        out_t[:], acc_t[:], UNSCALE, bias_col, ALU.mult, ALU.add
                    )

            xoff = [0, DB * C1]

            # ---------------- slot 0 (big expert, C1) ----------------
            # Head-latency critical: kick x[d0] + per-d w1 chunks as
            # separate small DMAs so the first matmul starts as soon as
            # ~0.3 MB has landed, not after the whole 2.3 MB.
            C = C1
            xa = xp.tile([P, C], f16, tag="xa")
            nc.sync.dma_start(xa[:], xT[:, 0:C])
            w1ts0 = []
            w1t = w1p.tile([P, H], f8, tag="w1")
            nc.sync.dma_start(w1t[:], w1s[0, :, 0:H])
            w1ts0.append(w1t)
            xb = xp.tile([P, (DB - 1) * C], f16, tag="xb")
            nc.sync.dma_start(xb[:], xT[:, C:DB * C])
            for d in range(1, DB):
                w1t = w1p.tile([P, H], f8, tag="w1")
                nc.sync.dma_start(w1t[:], w1s[0, :, d * H:(d + 1) * H])
                w1ts0.append(w1t)

            def rhs0(d):
                return xa[:] if d == 0 else xb[:, (d - 1) * C:d * C]

            # stage 1: two half-H passes, d-outer within each, so the PE
            # starts on w1[d0] immediately (second pass reuses resident
            # w1 tiles).
            for half in range(2):
                accs1 = [
                    ps.tile([P, C], f32, tag="acc", name=f"acc1_{half}_{i}")
                    for i in range(HB // 2)
                ]
                for d in range(DB):
                    for hh in range(HB // 2):
                        h = half * (HB // 2) + hh
                        nc.tensor.matmul(
                            accs1[hh][:],
                            lhsT=w1ts0[d][:, h * P:(h + 1) * P],
                            rhs=rhs0(d),
                            start=(d == 0),
                            stop=(d == DB - 1),
                        )
                for hh in range(HB // 2):
                    h = half * (HB // 2) + hh
                    ht = hp.tile([P, C], f16, tag="hT")
                    epi1(h, ht, accs1[hh], b1ts[0][:, h:h + 1])
                    hts[0][h] = ht

            # stage 2: h-outer so the PE consumes each w2 tile as soon
            # as its DMA lands.
            accs = [
                ps.tile([P, C], f32, tag="acc", name=f"acc2_0_{d}")
                for d in range(DB)
            ]
            for g in range(HB // W2G):
                w2t = w2p.tile([P, W2G * D], f8, tag="w2")
                nc.sync.dma_start(
                    w2t[:], w2s[0, :, g * W2G * D:(g + 1) * W2G * D]
                )
                for hh in range(W2G):
                    h = g * W2G + hh
                    for d in range(DB):
                        nc.tensor.matmul(
                            accs[d][:],
                            lhsT=w2t[:, hh * D + d * P:hh * D + d * P + P],
                            rhs=hts[0][h][:],
                            start=(h == 0),
                            stop=(h == HB - 1),
                        )
            yt0 = yp.tile([P, DB * C], f16, tag="yt")
            for d in range(DB):
                epi2(d, yt0[:, d * C:(d + 1) * C], accs[d],
                     b2ts[0][:, d:d + 1])
            # Slot 0's output rides the (otherwise idle) gpsimd queue so
            # it overlaps slot 1 compute without blocking weight loads.
            nc.gpsimd.dma_start(ys[0], yt0[:])

            # ---------------- slot 1 (small expert, C2) ----------------
            C = CS[1]
            xt = xp.tile([P, DB * C], f16, tag="xT")
            nc.sync.dma_start(xt[:], xT[:, xoff[1]:xoff[1] + DB * C])
            xts[1] = xt
            w1ts = []
            for g in range(DB // W1G):
                w1t = w1p.tile([P, W1G * H], f8, tag="w1")
                nc.sync.dma_start(
                    w1t[:], w1s[1, :, g * W1G * H:(g + 1) * W1G * H]
                )
                w1ts.append(w1t)
            # All of slot 1's w2 too: it is fully resident long before
            # stage 2 below reaches it (PE is the bottleneck by then).
            w2ts = []
            for g in range(HB // W2G):
                w2t = w2p.tile([P, W2G * D], f8, tag="w2")
                nc.sync.dma_start(
                    w2t[:], w2s[1, :, g * W2G * D:(g + 1) * W2G * D]
                )
                w2ts.append(w2t)

            # stage 1: h-outer retires each psum right away so the
            # epilogues pipeline with the next chain's matmuls.
            for h in range(HB):
                acc = ps.tile([P, C], f32, tag="acc")
                for d in range(DB):
                    w1t = w1ts[d // W1G]
                    col = (d % W1G) * H + h * P
                    nc.tensor.matmul(
                        acc[:],
                        lhsT=w1t[:, col:col + P],
                        rhs=xts[1][:, d * C:(d + 1) * C],
                        start=(d == 0),
                        stop=(d == DB - 1),
                    )
                ht = hp.tile([P, C], f16, tag="hT")
                epi1(h, ht, acc, b1ts[1][:, h:h + 1])
                hts[1][h] = ht

            # stage 2, d-blocked: each d-chunk runs its full h-chain,
            # retires its epilogue, and streams out in 2-chunk writes
            # while later chains still compute -- the tail after the
            # last matmul is one epilogue + one small DMA.
            yt1 = yp.tile([P, DB * C], f16, tag="yt")
            for d in range(DB):
                acc = ps.tile([P, C], f32, tag="acc", name=f"acc2_1_{d}")
                for h in range(HB):
                    g, hh = h // W2G, h % W2G
                    nc.tensor.matmul(
                        acc[:],
                        lhsT=w2ts[g][:, hh * D + d * P:hh * D + d * P + P],
                        rhs=hts[1][h][:],
                        start=(h == 0),
                        stop=(h == HB - 1),
                    )
                epi2(d, yt1[:, d * C:(d + 1) * C], acc, b2ts[1][:, d:d + 1])
                if d % 2 == 1:
                    nc.sync.dma_start(
                        ys[1][:, (d - 1) * C:(d + 1) * C],
                        yt1[:, (d - 1) * C:(d + 1) * C],
                    )

    nc.compile()
    return nc


def kernel(x, gate_w, gate_b, w1, b1, w2, b2, _trace=False):
    from concourse.bass_utils import run_bass_kernel_spmd

    f16 = np.float16
    e3m4 = ml_dtypes.float8_e3m4

    x = np.asarray(x, dtype=np.float32)
    B, S, d_in = x.shape
    T = B * S
    xf = x.reshape(T, d_in)

    # --- routing (host side: this is the dispatch/sharding step) ---
    logits = xf @ np.asarray(gate_w, dtype=np.float32) + np.asarray(
        gate_b, dtype=np.float32
    )
    top1 = np.argmax(logits, axis=-1)
    idxs = [np.nonzero(top1 == e)[0] for e in range(E)]
    counts = np.array([len(i) for i in idxs])

    # Pair big+small experts per core: slot 0 gets the 8 largest.
    order = np.argsort(-counts, kind="stable")
    slot_experts = [
        (int(order[core]), int(order[2 * NCORES - 1 - core]))
        for core in range(NCORES)
    ]

    def cap(n):
        return min(512, max(16, (n + 3) // 4 * 4))

    C1 = cap(max(counts[e0] for e0, _ in slot_experts))
    C2 = cap(max(counts[e1] for _, e1 in slot_experts))
    assert all(counts[a] <= C1 and counts[b] <= C2 for a, b in slot_experts)

    if (C1, C2) not in _program_cache:
        _program_cache[(C1, C2)] = _build_program(C1, C2)
    nc = _program_cache[(C1, C2)]

    # Pre-quantize all expert weights into their SBUF image layouts.
    w1q = (np.asarray(w1, dtype=np.float32) * S1).astype(e3m4)   # [E, D, H]
    w2q = (np.asarray(w2, dtype=np.float32) * S2).astype(e3m4)   # [E, H, D]
    b1f = np.asarray(b1, dtype=np.float32) * S1
    b2f = np.asarray(b2, dtype=np.float32)
    xf16 = xf.astype(f16)
    CS = (C1, C2)

    in_maps = []
    for core in range(NCORES):
        xT = np.zeros((P, DB * (C1 + C2)), dtype=f16)
        w1sv = np.empty((2, P, DB * H), dtype=e3m4)
        w2sv = np.empty((2, P, HB * D), dtype=e3m4)
        b1sv = np.empty((2, P, HB), dtype=np.float32)
        b2sv = np.empty((2, P, DB), dtype=np.float32)
        xoff = (0, DB * C1)
        for s in range(2):
            e = slot_experts[core][s]
            C = CS[s]
            idx = idxs[e]
            n = len(idx)
            if n:
                # [p, dd*C + c] = x[idx[c], dd*128 + p]
                xs = xf16[idx].T.reshape(DB, P, n).transpose(1, 0, 2)
                xv = xT[:, xoff[s]:xoff[s] + DB * C].reshape(P, DB, C)
                xv[:, :, :n] = xs
            # [p, dd*H + h] = w1q[e][dd*128 + p, h]
            w1sv[s] = (
                w1q[e].reshape(DB, P, H).transpose(1, 0, 2).reshape(P, DB * H)
            )
            # [p, oo*D + d] = w2q[e][oo*128 + p, d]
            w2sv[s] = (
                w2q[e].reshape(HB, P, D).transpose(1, 0, 2).reshape(P, HB * D)
            )
            b1sv[s] = b1f[e].reshape(HB, P).T
            b2sv[s] = b2f[e].reshape(DB, P).T
        in_maps.append(
            {"xT": xT, "w1s": w1sv, "w2s": w2sv, "b1s": b1sv, "b2s": b2sv}
        )

    res = run_bass_kernel_spmd(
        nc, in_maps, core_ids=list(range(NCORES)), trace=_trace
    )

    out = np.zeros((T, D), dtype=np.float32)
    for core in range(NCORES):
        for s, yname in ((0, "y0"), (1, "y1")):
            e = slot_experts[core][s]
            C = CS[s]
            idx = idxs[e]
            n = len(idx)
            if n:
                yv = res.results[core][yname]
                yd = (
                    yv.reshape(P, DB, C).transpose(1, 0, 2).reshape(D, C)
                )
                out[idx] = yd[:, :n].T.astype(np.float32)
    if _trace:
        kernel.last_result = res
    return out.reshape(B, S, D)
